# revision 5
# baseline (speedup 1.0000x reference)
"""2D Haar DWT (mode=0 'even') on Trainium2, 8 NeuronCores.

Input : x [2, 16, 16, 256, 256] f32, mode (0)
Output: [2, 64, 16, 128, 128] f32  (channel concat of LL, HL, LH, HH)

Sharding: the 2*16 = 32 (b, c) pairs are split 4-per-core across 8 cores.
Each core processes 4 groups x 16 depth-images of 256x256 and emits, for
each group, four subband stacks [16, 128, 128] that are contiguous slices
of the full output (y[b, s*16+c, :, :, :]). No inter-core communication.

Per-core kernel (Tile framework):
  - 8 iterations, each handling 8 depth-images (2 MiB in, 2 MiB out)
  - DMA in: tile [128, 4096]; partition p = row-pair, free = (d, 2, 256)
  - ACT scales by 0.5, DVE does the 2x2 butterfly:
      vs = even_row + odd_row          vd = odd_row - even_row
      LL = vs_even_col + vs_odd_col    HL = vs_odd_col - vs_even_col
      LH = vd_even_col + vd_odd_col    HH = vd_odd_col - vd_even_col
  - 4 DMAs out of [128, 1024] each (512 KiB contiguous DRAM chunks)
"""

import numpy as np

N_CORES = 8
B, C, D, H, W = 2, 16, 16, 256, 256
GROUPS_PER_CORE = 4  # (b,c) pairs per core
D_SPLIT = 2          # halves of the depth dim per group
D_SUB = D // D_SPLIT # images per iteration

_compiled_nc = None


def _build_nc():
    import concourse.bacc as bacc
    import concourse.tile as tile
    import concourse.mybir as mybir

    f32 = mybir.dt.float32
    nc = bacc.Bacc("TRN2", target_bir_lowering=False, debug=False,
                   num_devices=N_CORES)

    x = nc.dram_tensor("x", [GROUPS_PER_CORE, D, H, W], f32,
                       kind="ExternalInput")
    y = nc.dram_tensor("y", [GROUPS_PER_CORE, 4, D, H // 2, W // 2], f32,
                       kind="ExternalOutput")

    # [8, 128, 8, 2, 256]: iter, row-pair partition, d, row-parity, w
    xa = x.rearrange("g (i d) (p two) w -> (g i) p d two w",
                     i=D_SPLIT, d=D_SUB, two=2)
    # [4, 2, 4, 128, 8, 128]: group, half, subband, h, d, w
    ya = y.rearrange("bc s (i d) h w -> bc i s h d w",
                     i=D_SPLIT, d=D_SUB)

    n_iters = GROUPS_PER_CORE * D_SPLIT
    FD_IN = D_SUB * 2 * (W // 2) * 2   # 4096
    FD_MID = D_SUB * W                 # 2048
    FD_OUT = D_SUB * (W // 2)          # 1024

    with tile.TileContext(nc) as tc:
        with tc.tile_pool(name="io", bufs=2) as io_pool, \
             tc.tile_pool(name="mid", bufs=2) as mid_pool, \
             tc.tile_pool(name="outp", bufs=3) as out_pool:
            for it in range(n_iters):
                t_in = io_pool.tile([128, FD_IN], f32, tag="t_in")
                t_in_v = t_in[:].rearrange("p (d two w) -> p d two w",
                                           d=D_SUB, two=2)
                nc.sync.dma_start(t_in_v, xa[it])

                t_s = io_pool.tile([128, FD_IN], f32, tag="t_s")
                nc.scalar.mul(t_s[:], t_in[:], 0.5)

                tv = t_s[:].rearrange("p (d two w) -> p d two w",
                                      d=D_SUB, two=2)
                vs = mid_pool.tile([128, FD_MID], f32, tag="vs")
                vd = mid_pool.tile([128, FD_MID], f32, tag="vd")
                vs_v = vs[:].rearrange("p (d w) -> p d w", d=D_SUB)
                vd_v = vd[:].rearrange("p (d w) -> p d w", d=D_SUB)
                nc.vector.tensor_add(vs_v, tv[:, :, 0, :], tv[:, :, 1, :])
                nc.vector.tensor_sub(vd_v, tv[:, :, 1, :], tv[:, :, 0, :])

                sv = vs[:].rearrange("p (d w two) -> p d w two",
                                     d=D_SUB, two=2)
                dv = vd[:].rearrange("p (d w two) -> p d w two",
                                     d=D_SUB, two=2)

                subband_defs = [
                    ("ll", sv, False),  # LL = s_e + s_o
                    ("hl", sv, True),   # HL = s_o - s_e
                    ("lh", dv, False),  # LH = d_e + d_o
                    ("hh", dv, True),   # HH = d_o - d_e
                ]
                for s, (tag, src, is_sub) in enumerate(subband_defs):
                    ot = out_pool.tile([128, FD_OUT], f32, tag=tag)
                    ov = ot[:].rearrange("p (d w) -> p d w", d=D_SUB)
                    if is_sub:
                        nc.vector.tensor_sub(ov, src[:, :, :, 1],
                                             src[:, :, :, 0])
                    else:
                        nc.vector.tensor_add(ov, src[:, :, :, 0],
                                             src[:, :, :, 1])
                    nc.sync.dma_start(ya[it // D_SPLIT, it % D_SPLIT, s], ov)

    nc.compile()
    return nc


def _get_nc():
    global _compiled_nc
    if _compiled_nc is None:
        _compiled_nc = _build_nc()
    return _compiled_nc


def _haar_numpy(x):
    # mode='odd' fallback: pad one zero row/col at the end of H and W
    x = np.pad(x, ((0, 0), (0, 0), (0, 0), (0, 1), (0, 1)))
    x01 = x[:, :, :, 0::2, :] * 0.5
    x02 = x[:, :, :, 1::2, :] * 0.5
    x1 = x01[..., 0::2]
    x2 = x02[..., 0::2]
    x3 = x01[..., 1::2]
    x4 = x02[..., 1::2]
    return np.concatenate((x1 + x2 + x3 + x4, -x1 - x2 + x3 + x4,
                           -x1 + x2 - x3 + x4, x1 - x2 - x3 + x4), axis=1)


def run_device(in_maps, trace=False, **kwargs):
    """Run the compiled SPMD kernel; returns BassKernelResults."""
    from concourse.bass_utils import run_bass_kernel_spmd
    nc = _get_nc()
    return run_bass_kernel_spmd(nc, in_maps, core_ids=list(range(N_CORES)),
                                trace=trace, **kwargs)


def make_in_maps(x):
    xs = np.ascontiguousarray(np.asarray(x, dtype=np.float32)
                              .reshape(B * C, D, H, W))
    return [{"x": xs[GROUPS_PER_CORE * k: GROUPS_PER_CORE * (k + 1)]}
            for k in range(N_CORES)]


def gather_output(results):
    out = np.stack([results[k]["y"] for k in range(N_CORES)])
    # [8, 4, 4, 16, 128, 128] -> [b, c, s, d, h, w] -> [b, s*16+c, d, h, w]
    out = out.reshape(B, C, 4, D, H // 2, W // 2)
    out = out.transpose(0, 2, 1, 3, 4, 5).reshape(B, 4 * C, D,
                                                  H // 2, W // 2)
    return np.ascontiguousarray(out)


def kernel(x, mode):
    mode_val = int(np.asarray(mode))
    if mode_val != 0:
        return _haar_numpy(np.asarray(x, dtype=np.float32))
    res = run_device(make_in_maps(x))
    return gather_output(res.results)


# revision 14
# speedup vs baseline: 1.2348x; 1.2348x over previous
"""2D Haar DWT (mode=0 'even') on Trainium2, 8 NeuronCores.

Input : x [2, 16, 16, 256, 256] f32, mode (0)
Output: [2, 64, 16, 128, 128] f32  (channel concat of LL, HL, LH, HH)

Sharding: the 2*16 = 32 (b, c) pairs are split 4-per-core across 8 cores.
Each core processes 4 groups x 16 depth-images of 256x256 and emits, for
each group, four subband stacks [16, 128, 128] that are contiguous slices
of the full output (y[b, s*16+c, :, :, :]). No inter-core communication.

Per-core kernel (Tile framework), 8 iterations of 8 depth-images each:
  - partition p = (j, q): image j in [0,8) x 16-row block q in [0,16)
    so each partition holds 16 consecutive input rows (16 KiB contiguous
    DRAM per partition per input DMA) and produces 8 consecutive output
    rows (4 KiB contiguous DRAM per partition per output DMA).
  - input DMAs on the Sync HWDGE ring, output DMAs on the Scalar ring,
    0.5 prescale on GpSimd, 2x2 Haar butterfly on DVE:
      vs = even_row + odd_row          vd = odd_row - even_row
      LL = vs_even_col + vs_odd_col    HL = vs_odd_col - vs_even_col
      LH = vd_even_col + vd_odd_col    HH = vd_odd_col - vd_even_col
"""

import numpy as np

N_CORES = 8
B, C, D, H, W = 2, 16, 16, 256, 256
GROUPS_PER_CORE = 4  # (b,c) pairs per core
D_SPLIT = 2          # halves of the depth dim per group
D_SUB = D // D_SPLIT # images per iteration (8)

_compiled_nc = None


def _build_nc():
    import concourse.bacc as bacc
    import concourse.tile as tile
    import concourse.mybir as mybir

    f32 = mybir.dt.float32
    nc = bacc.Bacc("TRN2", target_bir_lowering=False, debug=False,
                   num_devices=N_CORES)

    x = nc.dram_tensor("x", [GROUPS_PER_CORE, D, H, W], f32,
                       kind="ExternalInput")
    y = nc.dram_tensor("y", [GROUPS_PER_CORE, 4, D, H // 2, W // 2], f32,
                       kind="ExternalOutput")

    # partition p = (j, q): image j (8), 16-row block q (16)
    # [8 iter, 128 part, 16 row, 256 w]; 16 KiB contiguous per partition
    xa = x.rearrange("g (i j) (q sixteen) w -> (g i) (j q) sixteen w",
                     i=D_SPLIT, j=D_SUB, q=16, sixteen=16)
    # output rows h = 8q + e; 4 KiB contiguous per partition
    # [4 grp, 2 half, 4 subband, 128 part, 8 e, 128 w]
    ya = y.rearrange("bc s (i j) (q e) w -> bc i s (j q) e w",
                     i=D_SPLIT, j=D_SUB, q=16, e=8)

    n_iters = GROUPS_PER_CORE * D_SPLIT
    FD_IN = 16 * W            # 4096
    FD_MID = 8 * W            # 2048
    FD_OUT = 8 * (W // 2)     # 1024

    with tile.TileContext(nc) as tc:
        with tc.tile_pool(name="io", bufs=3) as io_pool, \
             tc.tile_pool(name="mid", bufs=2) as mid_pool, \
             tc.tile_pool(name="outp", bufs=3) as out_pool:
            for it in range(n_iters):
                # last iteration in smaller row-chunks to shrink the
                # exposed compute tail after the final input lands
                chunks = [(0, 16)] if it < n_iters - 1 else \
                         [(0, 8), (8, 12), (12, 16)]
                for r0, r1 in chunks:
                    nr = r1 - r0
                    ne = nr // 2
                    t_in = io_pool.tile([128, nr * W], f32, tag="t_in")
                    t_in_v = t_in[:].rearrange("p (r w) -> p r w", r=nr)
                    nc.sync.dma_start(t_in_v, xa[it, :, r0:r1, :])

                    t_s = io_pool.tile([128, nr * W], f32, tag="t_s")
                    nc.scalar.mul(t_s[:], t_in[:], 0.5)

                    # rows r = 2e + par; even/odd row views [128, ne, 256]
                    tv = t_s[:].rearrange("p (e par w) -> p e par w",
                                          e=ne, par=2)
                    vs = mid_pool.tile([128, ne * W], f32, tag="vs")
                    vd = mid_pool.tile([128, ne * W], f32, tag="vd")
                    vs_v = vs[:].rearrange("p (e w) -> p e w", e=ne)
                    vd_v = vd[:].rearrange("p (e w) -> p e w", e=ne)
                    nc.vector.tensor_add(vs_v, tv[:, :, 0, :],
                                         tv[:, :, 1, :])
                    nc.vector.tensor_sub(vd_v, tv[:, :, 1, :],
                                         tv[:, :, 0, :])

                    # cols w = 2*w2 + par
                    sv = vs[:].rearrange("p (e w2 par) -> p e w2 par",
                                         e=ne, par=2)
                    dv = vd[:].rearrange("p (e w2 par) -> p e w2 par",
                                         e=ne, par=2)

                    subband_defs = [
                        ("ll", sv, False),  # LL = s_e + s_o
                        ("hl", sv, True),   # HL = s_o - s_e
                        ("lh", dv, False),  # LH = d_e + d_o
                        ("hh", dv, True),   # HH = d_o - d_e
                    ]
                    for s, (tag, src, is_sub) in enumerate(subband_defs):
                        ot = out_pool.tile([128, ne * (W // 2)], f32,
                                           tag=tag)
                        ov = ot[:].rearrange("p (e w2) -> p e w2", e=ne)
                        if is_sub:
                            nc.vector.tensor_sub(ov, src[:, :, :, 1],
                                                 src[:, :, :, 0])
                        else:
                            nc.vector.tensor_add(ov, src[:, :, :, 0],
                                                 src[:, :, :, 1])
                        dma_eng = nc.scalar if s < 2 else nc.sync
                        dma_eng.dma_start(
                            ya[it // D_SPLIT, it % D_SPLIT, s]
                              [:, r0 // 2:r1 // 2, :], ov)

    nc.compile()
    return nc


def _get_nc():
    global _compiled_nc
    if _compiled_nc is None:
        _compiled_nc = _build_nc()
    return _compiled_nc


def _haar_numpy(x):
    # mode='odd' fallback: pad one zero row/col at the end of H and W
    x = np.pad(x, ((0, 0), (0, 0), (0, 0), (0, 1), (0, 1)))
    x01 = x[:, :, :, 0::2, :] * 0.5
    x02 = x[:, :, :, 1::2, :] * 0.5
    x1 = x01[..., 0::2]
    x2 = x02[..., 0::2]
    x3 = x01[..., 1::2]
    x4 = x02[..., 1::2]
    return np.concatenate((x1 + x2 + x3 + x4, -x1 - x2 + x3 + x4,
                           -x1 + x2 - x3 + x4, x1 - x2 - x3 + x4), axis=1)


def run_device(in_maps, trace=False, **kwargs):
    """Run the compiled SPMD kernel; returns BassKernelResults."""
    from concourse.bass_utils import run_bass_kernel_spmd
    nc = _get_nc()
    return run_bass_kernel_spmd(nc, in_maps, core_ids=list(range(N_CORES)),
                                trace=trace, **kwargs)


def make_in_maps(x):
    xs = np.ascontiguousarray(np.asarray(x, dtype=np.float32)
                              .reshape(B * C, D, H, W))
    return [{"x": xs[GROUPS_PER_CORE * k: GROUPS_PER_CORE * (k + 1)]}
            for k in range(N_CORES)]


def gather_output(results):
    out = np.stack([results[k]["y"] for k in range(N_CORES)])
    # [8, 4, 4, 16, 128, 128] -> [b, c, s, d, h, w] -> [b, s*16+c, d, h, w]
    out = out.reshape(B, C, 4, D, H // 2, W // 2)
    out = out.transpose(0, 2, 1, 3, 4, 5).reshape(B, 4 * C, D,
                                                  H // 2, W // 2)
    return np.ascontiguousarray(out)


def kernel(x, mode):
    mode_val = int(np.asarray(mode))
    if mode_val != 0:
        return _haar_numpy(np.asarray(x, dtype=np.float32))
    in_maps = make_in_maps(x)
    try:
        res = run_device(in_maps)
    except Exception:
        res = run_device(in_maps)  # one retry for transient device errors
    return gather_output(res.results)
